# revision 16
# baseline (speedup 1.0000x reference)
"""Batched multi-head attention kernel for Trainium2 (Bass/Tile) — v4.

Problem: q,k,v [256, 16, 49, 64] fp32 -> out [256, 16, 49, 64] fp32
  s = (q @ k^T) / sqrt(64); p = exp(s - max) / (sum exp + 1e-9); out = p @ v

Sharding: data-parallel over B across 8 NeuronCores (512 independent (b,h)
"pairs" per core). No communication.

v4 redesign vs v3 (DMA-bound at ~200us; SP queue 92% busy on serial DMA):
 - ALL input loads are Pool-engine (SWDGE) cast-DMAs fp32->bf16. The DMA
   transfer cost is charged on OUTPUT bytes, so casting in flight halves
   the serial DMA_ENGINES time, and the separate Pool cast ops vanish.
     q/k: pair-flat [pair, 49*64] bf16 (6.1KB/partition descriptors).
     v:   directly into [k, pair, d] matmul layout (128B runs, 2x penalty,
          but bf16 halves the bytes vs the v3 fp32 strided load).
 - Output DRAM layout is [q, pair, d] (was [pair, q, d]): stores from the
   natural [q-partition, pair, d] SBUF tile become 8KB-contiguous
   descriptors (4x fewer DMA-ns than v3's 128B-run stores). The host
   transposes back during unshard (it already materializes for fp32 cast).
 - GROUP=16 pairs (was 8) with NPS=2 PSUM rotation: halves the count of
   exp/recip/normalize ops whose fixed PSUM-access latencies dominated.
 - normalize alternates DVE/ACT per group to balance engine busy time.
 - loads for all superblocks are issued upfront (everything persists in
   SBUF; no buffer rotation) so the serial DMA pipe never starves.
"""

import sys

for _p in ("/opt/trn_rl_repo", "/opt/pypackages"):
    if _p not in sys.path:
        sys.path.insert(0, _p)

import contextlib

import numpy as np

import concourse.bacc as bacc
import concourse.bass as bass
import concourse.tile as tile
from concourse import mybir
from concourse.bass_utils import run_bass_kernel_spmd

B, H, NQ, NK, D = 256, 16, 49, 49, 64
N_CORES = 8
PAIRS_PER_CORE = (B // N_CORES) * H  # 512
GROUP = 16  # pairs per compute group
SCALE = float(1.0 / np.sqrt(D))
NPS = 2  # PSUM buffer sets (s_ps + o_ps = 4KB+4KB = 4 banks per set)

F32 = mybir.dt.float32
BF16 = mybir.dt.bfloat16


def _superblock_sizes(npairs: int):
    """Superblock sizes in pairs; each is 64 or 128."""
    assert npairs % 64 == 0
    sizes = []
    rem = npairs
    while rem >= 128:
        sizes.append(128)
        rem -= 128
    if rem:
        sizes.append(rem)
    return sizes


def build_nc(npairs: int = PAIRS_PER_CORE, repeats: int = 1):
    """repeats > 1 wraps the computation in a dynamic loop recomputing the
    identical outputs; used only for wall-clock slope timing."""
    nc = bacc.Bacc("TRN2", target_bir_lowering=False, debug=False)

    qd = nc.dram_tensor("q", [npairs * NQ, D], F32, kind="ExternalInput")
    kd = nc.dram_tensor("k", [npairs * NK, D], F32, kind="ExternalInput")
    vd = nc.dram_tensor("v", [npairs * NK, D], F32, kind="ExternalInput")
    # out is [q, pair, d] so stores are 8KB-contiguous; host transposes back.
    od = nc.dram_tensor("out", [NQ * npairs, D], BF16, kind="ExternalOutput")

    sizes = _superblock_sizes(npairs)
    nsb = len(sizes)
    bases = [sum(sizes[:i]) for i in range(nsb)]

    with tile.TileContext(nc) as tc:
        with (
            tc.tile_pool(name="small", bufs=4) as small,
            tc.tile_pool(name="persist", bufs=1) as persist,
            tc.tile_pool(name="persist_ps", bufs=1, space="PSUM") as persist_ps,
        ):
            # PSUM buffer sets (manual rotation): junk rows memset ONCE.
            s_ps_bufs, o_ps_bufs = [], []
            for i in range(NPS):
                s_ps_bufs.append(
                    persist_ps.tile(
                        [128, 8, 2, 64], F32, name=f"s_ps{i}", tag=f"s_ps{i}"
                    )
                )
                o_ps_bufs.append(
                    persist_ps.tile([128, 8, 128], F32, name=f"o_ps{i}", tag=f"o_ps{i}")
                )
            for t in s_ps_bufs:
                nc.vector.memset(t[:, :, :, :], 0.0)
            for t in o_ps_bufs:
                nc.vector.memset(t[:, :, :], 1.0)

            # Per-superblock persistent SBUF tiles (no rotation; all fit).
            qb_t, kb_t, qT_t, kT_t, out_t = [], [], [], [], []
            for i, sz in enumerate(sizes):
                half = sz // 2
                qb_t.append(persist.tile([sz, NQ * D], BF16, name=f"qb{i}", tag=f"qb{i}"))
                kb_t.append(persist.tile([sz, NQ * D], BF16, name=f"kb{i}", tag=f"kb{i}"))
                qT_t.append(
                    persist.tile([sz, NQ, 2, 32], BF16, name=f"qT{i}", tag=f"qT{i}")
                )
                kT_t.append(
                    persist.tile([sz, NQ, 2, 32], BF16, name=f"kT{i}", tag=f"kT{i}")
                )
                out_t.append(
                    persist.tile([113, half, D], BF16, name=f"ob{i}", tag=f"ob{i}")
                )
            # v path: pair-flat bf16 cast-load (big descriptors) into
            # [pair][(k64, d)] with junk k rows 49:64, then a DVE stream
            # transpose with a (d, khi, klo)-ordered input view producing the
            # WRAPPED layout vT[32*pb + k%32][d, k//32, pw].
            vf_t = []
            for i, sz in enumerate(sizes):
                vf_t.append(
                    persist.tile([sz, 64 * D], BF16, name=f"vf{i}", tag=f"vf{i}")
                )
            for t in vf_t:
                nc.gpsimd.memset(t[:, NK * D :], 0.0)  # junk k rows
            vr_bufs, vT_bufs = [], []
            for i in range(2):
                vr_bufs.append(
                    persist.tile([128, D, 64], BF16, name=f"vr{i}", tag=f"vr{i}")
                )
                vT_bufs.append(
                    persist.tile([128, D + 1, 2, 32], BF16, name=f"vT{i}", tag=f"vT{i}")
                )
            for t in vT_bufs:
                # ones "d=64" row for l = sum_k e
                nc.gpsimd.memset(t[:, D : D + 1, :, :], 1.0)

            rep_ctx = (
                tc.For_i(
                    0,
                    repeats,
                    1,
                    hint_engines=(
                        mybir.EngineType.PE,
                        mybir.EngineType.Activation,
                        mybir.EngineType.DVE,
                        mybir.EngineType.SP,
                        mybir.EngineType.Pool,
                    ),
                )
                if repeats > 1
                else contextlib.nullcontext()
            )
            with rep_ctx:
                # ---- all loads upfront (Pool SWDGE cast-DMAs fp32->bf16) ----
                for i, sz in enumerate(sizes):
                    r0 = bases[i] * NQ
                    nc.gpsimd.dma_start(
                        out=qb_t[i][:],
                        in_=qd[r0 : r0 + sz * NQ, :].rearrange(
                            "(p r) d -> p (r d)", p=sz
                        ),
                    )
                    nc.gpsimd.dma_start(
                        out=kb_t[i][:],
                        in_=kd[r0 : r0 + sz * NQ, :].rearrange(
                            "(p r) d -> p (r d)", p=sz
                        ),
                    )
                    nc.gpsimd.dma_start(
                        out=vf_t[i][0:sz, 0 : NK * D],
                        in_=vd[r0 : r0 + sz * NK, :].rearrange(
                            "(p r) d -> p (r d)", p=sz
                        ),
                    )

                gctr = 0
                for i, sz in enumerate(sizes):
                    base = bases[i]
                    nb = sz // 32  # partition blocks of pairs (2 or 4)
                    nh = nb // 2  # pb-block parity classes
                    half = sz // 2
                    ngroups = sz // GROUP
                    qb, kb, vf = qb_t[i], kb_t[i], vf_t[i]
                    qT, kT, out_sb = qT_t[i], kT_t[i], out_t[i]
                    vT = vT_bufs[i % 2]

                    # ---- transposes (DVE stream transpose) ----
                    nc.vector.transpose(qT[:], qb[:])
                    nc.vector.transpose(kT[:], kb[:])
                    # v wrap: ACT flip [pair][(k,d)] -> [pair][(d,k)], then a
                    # plain stream transpose of the (d, khi, klo)-contiguous
                    # tile: vT[32pb + klo][d, khi, pw] = v[32pb+pw][32khi+klo, d]
                    vr = vr_bufs[i % 2]
                    vf_ap = vf[0:sz, :]
                    vf_dk = bass.AP(
                        vf_ap.tensor, vf_ap.offset, [vf_ap.ap[0], [1, D], [D, 64]]
                    )
                    nc.scalar.copy(out=vr[0:sz, :, :], in_=vf_dk)
                    nc.vector.transpose(vT[0:sz, 0:D, :, :], vr[0:sz, :, :])

                    for g in range(ngroups):
                        h = g % nh
                        gt = g // nh
                        s_ps = s_ps_bufs[gctr % NPS]
                        o_ps = o_ps_bufs[gctr % NPS]
                        gctr += 1

                        # ---- score matmuls (wrapped out layout) ----
                        # s_ps[32pb + klo, u, khi, q], M split at the klo=32
                        # boundary; K=32 dhi accumulation as before.
                        for u in range(8):
                            pw = 8 * gt + u
                            for e in (0, 1):
                                pb = h + e * nh
                                for khi, mk in ((0, 32), (1, NK - 32)):
                                    for dhi in range(2):
                                        nc.tensor.matmul(
                                            s_ps[
                                                32 * pb : 32 * pb + mk, u, khi, 0:NQ
                                            ],
                                            kT[
                                                32 * pb : 32 * pb + 32,
                                                32 * khi : 32 * khi + mk,
                                                dhi,
                                                pw,
                                            ],
                                            qT[32 * pb : 32 * pb + 32, :, dhi, pw],
                                            start=(dhi == 0),
                                            stop=(dhi == 1),
                                            tile_position=(32 * pb, 32 * pb),
                                        )

                        # ---- exp (one ACT op; scale folded in) ----
                        eT = small.tile([128, 8, 2, NQ], BF16, tag="eT")
                        nc.scalar.activation(
                            out=eT[:],
                            in_=s_ps[0:128, :, :, 0:NQ],
                            func=mybir.ActivationFunctionType.Exp,
                            scale=SCALE,
                        )

                        # ---- out matmuls: o[q, 0:65] = sum_khi eT.T @ [v|1] ----
                        for u in range(8):
                            pw = 8 * gt + u
                            for e in (0, 1):
                                pb = h + e * nh
                                po = slice(64 * e, 64 * e + NQ)
                                for khi, mk in ((0, 32), (1, NK - 32)):
                                    nc.tensor.matmul(
                                        o_ps[po, u, 0 : D + 1],
                                        eT[32 * pb : 32 * pb + mk, u, khi, :],
                                        vT[32 * pb : 32 * pb + mk, :, khi, pw],
                                        start=(khi == 0),
                                        stop=(khi == 1),
                                        tile_position=(32 * pb, 64 * e),
                                    )

                        # ---- normalize: out = outU * (1/l) ----
                        r_t = small.tile([113, 8], F32, tag="r_t")
                        nc.vector.reciprocal(r_t[0:113, :], o_ps[0:113, :, D])
                        r_ap = r_t[0:113, :]
                        r_b = bass.AP(r_ap.tensor, r_ap.offset, r_ap.ap + [[0, D]])
                        col0 = (h * (ngroups // nh) + gt) * 8
                        nc.vector.tensor_mul(
                            out_sb[0:113, col0 : col0 + 8, :],
                            o_ps[0:113, :, 0:D],
                            r_b,
                        )

                    # ---- stores: od[q, p, d]; 8KB runs; one DMA per half ----
                    for e in (0, 1):
                        p0 = base + e * half
                        dst = bass.AP(
                            od, p0 * D, [[npairs * D, NQ], [D, half], [1, D]]
                        )
                        nc.sync.dma_start(
                            out=dst, in_=out_sb[64 * e : 64 * e + NQ, 0:half, :]
                        )

    nc.compile()
    return nc


_NC_CACHE: dict = {}


def _get_nc(npairs: int = PAIRS_PER_CORE, repeats: int = 1):
    key = (npairs, repeats)
    if key not in _NC_CACHE:
        _NC_CACHE[key] = build_nc(npairs, repeats)
    return _NC_CACHE[key]


def run_sharded(q, k, v, trace=False, **spmd_kwargs):
    """q,k,v: full [B, H, NQ/NK, D] fp32 arrays. Returns (out, results)."""
    q = np.ascontiguousarray(np.asarray(q, dtype=np.float32))
    k = np.ascontiguousarray(np.asarray(k, dtype=np.float32))
    v = np.ascontiguousarray(np.asarray(v, dtype=np.float32))
    bs = B // N_CORES
    in_maps = []
    for i in range(N_CORES):
        sl = slice(i * bs, (i + 1) * bs)
        in_maps.append(
            {
                "q": q[sl].reshape(PAIRS_PER_CORE * NQ, D),
                "k": k[sl].reshape(PAIRS_PER_CORE * NK, D),
                "v": v[sl].reshape(PAIRS_PER_CORE * NK, D),
            }
        )
    nc = _get_nc()
    res = run_bass_kernel_spmd(
        nc, in_maps, list(range(N_CORES)), trace=trace, **spmd_kwargs
    )
    outs = [
        np.asarray(res.results[i]["out"])
        .astype(np.float32)
        .reshape(NQ, PAIRS_PER_CORE, D)
        .transpose(1, 0, 2)
        .reshape(bs, H, NQ, D)
        for i in range(N_CORES)
    ]
    full = np.concatenate(outs, axis=0)
    return full, res


def kernel(q, k, v):
    out, _ = run_sharded(q, k, v, trace=False)
    return out


if __name__ == "__main__":
    # CoreSim smoke test on small variants.
    from concourse.bass_interp import CoreSim

    for npairs in (64, 192):
        nc = build_nc(npairs)
        rng = np.random.default_rng(0)
        q = rng.standard_normal((npairs * NQ, D)).astype(np.float32)
        k = rng.standard_normal((npairs * NK, D)).astype(np.float32)
        v = rng.standard_normal((npairs * NK, D)).astype(np.float32)

        sim = CoreSim(nc)
        sim.tensor("q")[:] = q
        sim.tensor("k")[:] = k
        sim.tensor("v")[:] = v
        sim.simulate()
        got = (
            np.array(sim.tensor("out"))
            .astype(np.float32)
            .reshape(NQ, npairs, D)
            .transpose(1, 0, 2)
        )

        s = np.einsum(
            "pqd,pkd->pqk", q.reshape(npairs, NQ, D), k.reshape(npairs, NK, D)
        )
        s *= SCALE
        m = s.max(-1, keepdims=True)
        e = np.exp(s - m)
        p = e / (e.sum(-1, keepdims=True) + 1e-9)
        want = np.einsum("pqk,pkd->pqd", p, v.reshape(npairs, NK, D))

        err = np.abs(got - want)
        print(f"npairs={npairs}")
        print("  absmax err:", err.max())
        print("  absmax-rel:", err.max() / np.abs(want).max())
        print("  L2 rel:", np.linalg.norm(got - want) / np.linalg.norm(want))


# revision 18
# speedup vs baseline: 1.2740x; 1.2740x over previous
"""Batched multi-head attention kernel for Trainium2 (Bass/Tile) — v4.

Problem: q,k,v [256, 16, 49, 64] fp32 -> out [256, 16, 49, 64] fp32
  s = (q @ k^T) / sqrt(64); p = exp(s - max) / (sum exp + 1e-9); out = p @ v

Sharding: data-parallel over B across 8 NeuronCores (512 independent (b,h)
"pairs" per core). No communication.

v4 redesign vs v3 (DMA-bound at ~200us; SP queue 92% busy on serial DMA):
 - ALL input loads are Pool-engine (SWDGE) cast-DMAs fp32->bf16. The DMA
   transfer cost is charged on OUTPUT bytes, so casting in flight halves
   the serial DMA_ENGINES time, and the separate Pool cast ops vanish.
     q/k: pair-flat [pair, 49*64] bf16 (6.1KB/partition descriptors).
     v:   directly into [k, pair, d] matmul layout (128B runs, 2x penalty,
          but bf16 halves the bytes vs the v3 fp32 strided load).
 - Output DRAM layout is [q, pair, d] (was [pair, q, d]): stores from the
   natural [q-partition, pair, d] SBUF tile become 8KB-contiguous
   descriptors (4x fewer DMA-ns than v3's 128B-run stores). The host
   transposes back during unshard (it already materializes for fp32 cast).
 - GROUP=16 pairs (was 8) with NPS=2 PSUM rotation: halves the count of
   exp/recip/normalize ops whose fixed PSUM-access latencies dominated.
 - normalize alternates DVE/ACT per group to balance engine busy time.
 - loads for all superblocks are issued upfront (everything persists in
   SBUF; no buffer rotation) so the serial DMA pipe never starves.
"""

import sys

for _p in ("/opt/trn_rl_repo", "/opt/pypackages"):
    if _p not in sys.path:
        sys.path.insert(0, _p)

import contextlib

import numpy as np

import concourse.bacc as bacc
import concourse.bass as bass
import concourse.tile as tile
from concourse import mybir
from concourse.bass_utils import run_bass_kernel_spmd

B, H, NQ, NK, D = 256, 16, 49, 49, 64
N_CORES = 8
PAIRS_PER_CORE = (B // N_CORES) * H  # 512
GROUP = 16  # pairs per compute group
SCALE = float(1.0 / np.sqrt(D))
NPS = 2  # PSUM buffer sets (s_ps + o_ps = 4KB+4KB = 4 banks per set)

F32 = mybir.dt.float32
BF16 = mybir.dt.bfloat16


def _superblock_sizes(npairs: int):
    """Superblock sizes in pairs (64 or 128). First/last are 64 so pipeline
    fill (first q/k load + transpose gate PE) and drain are cheap."""
    assert npairs % 64 == 0
    n64 = npairs // 64
    if n64 >= 4 and n64 % 2 == 0:
        return [64] + [128] * ((n64 - 2) // 2) + [64]
    sizes = []
    rem = npairs
    while rem >= 128:
        sizes.append(128)
        rem -= 128
    if rem:
        sizes.append(rem)
    return sizes


def build_nc(npairs: int = PAIRS_PER_CORE, repeats: int = 1):
    """repeats > 1 wraps the computation in a dynamic loop recomputing the
    identical outputs; used only for wall-clock slope timing."""
    nc = bacc.Bacc("TRN2", target_bir_lowering=False, debug=False)

    qd = nc.dram_tensor("q", [npairs * NQ, D], F32, kind="ExternalInput")
    kd = nc.dram_tensor("k", [npairs * NK, D], F32, kind="ExternalInput")
    vd = nc.dram_tensor("v", [npairs * NK, D], F32, kind="ExternalInput")
    # out is [q, pair, d] so stores are 8KB-contiguous; host transposes back.
    od = nc.dram_tensor("out", [NQ * npairs, D], BF16, kind="ExternalOutput")

    sizes = _superblock_sizes(npairs)
    nsb = len(sizes)
    bases = [sum(sizes[:i]) for i in range(nsb)]

    with tile.TileContext(nc) as tc:
        with (
            tc.tile_pool(name="small", bufs=4) as small,
            tc.tile_pool(name="persist", bufs=1) as persist,
            tc.tile_pool(name="persist_ps", bufs=1, space="PSUM") as persist_ps,
        ):
            # PSUM buffer sets (manual rotation): junk rows memset ONCE.
            s_ps_bufs, o_ps_bufs = [], []
            for i in range(NPS):
                s_ps_bufs.append(
                    persist_ps.tile([128, 8, 128], F32, name=f"s_ps{i}", tag=f"s_ps{i}")
                )
                o_ps_bufs.append(
                    persist_ps.tile([128, 8, 128], F32, name=f"o_ps{i}", tag=f"o_ps{i}")
                )
            for t in s_ps_bufs:
                nc.vector.memset(t[32:64, :, :], 0.0)
            for t in o_ps_bufs:
                nc.vector.memset(t[32:64, :, :], 1.0)

            # Per-superblock persistent SBUF tiles (no rotation; all fit).
            qb_t, kb_t, qT_t, kT_t, out_t = [], [], [], [], []
            for i, sz in enumerate(sizes):
                half = sz // 2
                qb_t.append(persist.tile([sz, NQ * D], BF16, name=f"qb{i}", tag=f"qb{i}"))
                kb_t.append(persist.tile([sz, NQ * D], BF16, name=f"kb{i}", tag=f"kb{i}"))
                qT_t.append(
                    persist.tile([sz, NQ, 2, 32], BF16, name=f"qT{i}", tag=f"qT{i}")
                )
                kT_t.append(
                    persist.tile([sz, NQ, 2, 32], BF16, name=f"kT{i}", tag=f"kT{i}")
                )
                out_t.append(
                    persist.tile([113, half, D], BF16, name=f"ob{i}", tag=f"ob{i}")
                )
            # v double-buffers: fp32 strided-load target + bf16 cast target.
            v_sb_bufs, vb_bufs = [], []
            for i in range(2):
                v_sb_bufs.append(
                    persist.tile([113, 64, D], F32, name=f"v_sb{i}", tag=f"v_sb{i}")
                )
                vb_bufs.append(
                    persist.tile([113, 64, D + 1], BF16, name=f"vbb{i}", tag=f"vbb{i}")
                )
            for t in v_sb_bufs:
                nc.gpsimd.memset(t[32:64, :, :], 0.0)
            for t in vb_bufs:
                # junk k rows (49:64) zero; ones column for l = sum_k e.
                nc.gpsimd.memset(t[32:64, :, :], 0.0)
                nc.gpsimd.memset(t[0:113, :, D : D + 1], 1.0)

            rep_ctx = (
                tc.For_i(
                    0,
                    repeats,
                    1,
                    hint_engines=(
                        mybir.EngineType.PE,
                        mybir.EngineType.Activation,
                        mybir.EngineType.DVE,
                        mybir.EngineType.SP,
                        mybir.EngineType.Pool,
                    ),
                )
                if repeats > 1
                else contextlib.nullcontext()
            )
            with rep_ctx:
                # ---- q/k loads upfront (Pool SWDGE cast-DMAs fp32->bf16) ----
                for i, sz in enumerate(sizes):
                    r0 = bases[i] * NQ
                    nc.gpsimd.dma_start(
                        out=qb_t[i][:],
                        in_=qd[r0 : r0 + sz * NQ, :].rearrange(
                            "(p r) d -> p (r d)", p=sz
                        ),
                    )
                    nc.gpsimd.dma_start(
                        out=kb_t[i][:],
                        in_=kd[r0 : r0 + sz * NQ, :].rearrange(
                            "(p r) d -> p (r d)", p=sz
                        ),
                    )

                # v loads: HWDGE fp32 strided, 1-superblock-ahead prefetch into
                # rotating bufs. Must be emitted AFTER the previous user of the
                # target buffer (forward-only dependency tracking).
                def load_v(i):
                    sz = sizes[i]
                    base = bases[i]
                    half = sz // 2
                    v_sb = v_sb_bufs[i % 2]
                    for e, eng in ((0, nc.sync), (1, nc.scalar)):
                        rows = vd[
                            (base + e * half) * NK : (base + (e + 1) * half) * NK, :
                        ]
                        eng.dma_start(
                            out=v_sb[64 * e : 64 * e + NK, 0:half, :],
                            in_=rows.rearrange("(p r) d -> r p d", p=half),
                        )

                load_v(0)
                gctr = 0
                for i, sz in enumerate(sizes):
                    base = bases[i]
                    nb = sz // 32  # partition blocks of pairs (2 or 4)
                    nh = nb // 2  # pb-block parity classes
                    half = sz // 2
                    ngroups = sz // GROUP
                    qb, kb = qb_t[i], kb_t[i]
                    qT, kT, out_sb = qT_t[i], kT_t[i], out_t[i]
                    v_sb, vb = v_sb_bufs[i % 2], vb_bufs[i % 2]

                    # ---- v cast fp32->bf16 (ACT, contiguous) ----
                    nc.scalar.copy(
                        out=vb[0:113, 0:half, 0:D], in_=v_sb[0:113, 0:half, :]
                    )
                    if i + 1 < len(sizes):
                        load_v(i + 1)

                    # ---- transposes (DVE stream transpose) ----
                    nc.vector.transpose(qT[:], qb[:])
                    nc.vector.transpose(kT[:], kb[:])

                    for g in range(ngroups):
                        h = g % nh
                        gt = g // nh
                        s_ps = s_ps_bufs[gctr % NPS]
                        o_ps = o_ps_bufs[gctr % NPS]
                        gctr += 1

                        # ---- score matmuls: sT[key, q], 2 x K=32 accum ----
                        for u in range(8):
                            pw = 8 * gt + u
                            for e in (0, 1):
                                pb = h + e * nh
                                po = slice(64 * e, 64 * e + NQ)
                                for dhi in range(2):
                                    nc.tensor.matmul(
                                        s_ps[po, u, 0:NQ],
                                        kT[32 * pb : 32 * pb + 32, :, dhi, pw],
                                        qT[32 * pb : 32 * pb + 32, :, dhi, pw],
                                        start=(dhi == 0),
                                        stop=(dhi == 1),
                                        tile_position=(32 * pb, 64 * e),
                                    )

                        # ---- exp (one ACT op; scale folded in) ----
                        eT = small.tile([113, 8, NQ], BF16, tag="eT")
                        nc.scalar.activation(
                            out=eT[:],
                            in_=s_ps[0:113, :, 0:NQ],
                            func=mybir.ActivationFunctionType.Exp,
                            scale=SCALE,
                        )

                        # ---- out matmuls: outU[q, 0:65] = eT.T @ [v | 1] ----
                        for u in range(8):
                            pw = 8 * gt + u
                            for e in (0, 1):
                                po = slice(64 * e, 64 * e + NQ)
                                j = 32 * h + pw if nb == 4 else pw
                                nc.tensor.matmul(
                                    o_ps[po, u, 0 : D + 1],
                                    eT[po, u, :],
                                    vb[po, j, :],
                                    tile_position=(64 * e, 64 * e),
                                )

                        # ---- normalize: out = outU * (1/l) ----
                        r_t = small.tile([113, 8], F32, tag="r_t")
                        nc.vector.reciprocal(r_t[0:113, :], o_ps[0:113, :, D])
                        r_ap = r_t[0:113, :]
                        r_b = bass.AP(r_ap.tensor, r_ap.offset, r_ap.ap + [[0, D]])
                        col0 = (h * (ngroups // nh) + gt) * 8
                        nc.vector.tensor_mul(
                            out_sb[0:113, col0 : col0 + 8, :],
                            o_ps[0:113, :, 0:D],
                            r_b,
                        )

                    # ---- stores: od[q, p, d]; 8KB runs; one DMA per half ----
                    for e in (0, 1):
                        p0 = base + e * half
                        dst = bass.AP(
                            od, p0 * D, [[npairs * D, NQ], [D, half], [1, D]]
                        )
                        nc.sync.dma_start(
                            out=dst, in_=out_sb[64 * e : 64 * e + NQ, 0:half, :]
                        )

    nc.compile()
    return nc


_NC_CACHE: dict = {}


def _get_nc(npairs: int = PAIRS_PER_CORE, repeats: int = 1):
    key = (npairs, repeats)
    if key not in _NC_CACHE:
        _NC_CACHE[key] = build_nc(npairs, repeats)
    return _NC_CACHE[key]


def run_sharded(q, k, v, trace=False, **spmd_kwargs):
    """q,k,v: full [B, H, NQ/NK, D] fp32 arrays. Returns (out, results)."""
    q = np.ascontiguousarray(np.asarray(q, dtype=np.float32))
    k = np.ascontiguousarray(np.asarray(k, dtype=np.float32))
    v = np.ascontiguousarray(np.asarray(v, dtype=np.float32))
    bs = B // N_CORES
    in_maps = []
    for i in range(N_CORES):
        sl = slice(i * bs, (i + 1) * bs)
        in_maps.append(
            {
                "q": q[sl].reshape(PAIRS_PER_CORE * NQ, D),
                "k": k[sl].reshape(PAIRS_PER_CORE * NK, D),
                "v": v[sl].reshape(PAIRS_PER_CORE * NK, D),
            }
        )
    nc = _get_nc()
    res = run_bass_kernel_spmd(
        nc, in_maps, list(range(N_CORES)), trace=trace, **spmd_kwargs
    )
    outs = [
        np.asarray(res.results[i]["out"])
        .astype(np.float32)
        .reshape(NQ, PAIRS_PER_CORE, D)
        .transpose(1, 0, 2)
        .reshape(bs, H, NQ, D)
        for i in range(N_CORES)
    ]
    full = np.concatenate(outs, axis=0)
    return full, res


def kernel(q, k, v):
    out, _ = run_sharded(q, k, v, trace=False)
    return out


if __name__ == "__main__":
    # CoreSim smoke test on small variants.
    from concourse.bass_interp import CoreSim

    for npairs in (64, 192):
        nc = build_nc(npairs)
        rng = np.random.default_rng(0)
        q = rng.standard_normal((npairs * NQ, D)).astype(np.float32)
        k = rng.standard_normal((npairs * NK, D)).astype(np.float32)
        v = rng.standard_normal((npairs * NK, D)).astype(np.float32)

        sim = CoreSim(nc)
        sim.tensor("q")[:] = q
        sim.tensor("k")[:] = k
        sim.tensor("v")[:] = v
        sim.simulate()
        got = (
            np.array(sim.tensor("out"))
            .astype(np.float32)
            .reshape(NQ, npairs, D)
            .transpose(1, 0, 2)
        )

        s = np.einsum(
            "pqd,pkd->pqk", q.reshape(npairs, NQ, D), k.reshape(npairs, NK, D)
        )
        s *= SCALE
        m = s.max(-1, keepdims=True)
        e = np.exp(s - m)
        p = e / (e.sum(-1, keepdims=True) + 1e-9)
        want = np.einsum("pqk,pkd->pqd", p, v.reshape(npairs, NK, D))

        err = np.abs(got - want)
        print(f"npairs={npairs}")
        print("  absmax err:", err.max())
        print("  absmax-rel:", err.max() / np.abs(want).max())
        print("  L2 rel:", np.linalg.norm(got - want) / np.linalg.norm(want))
